# revision 49
# baseline (speedup 1.0000x reference)
"""Chamfer distance (squared L2) on 8 Trainium2 NeuronCores.

Problem: xyz1 [16, 4096, 3], xyz2 [16, 4096, 3] fp32.
  d[b,n,m] = ||xyz1[b,n] - xyz2[b,m]||^2
  out = (mean_{b,n} min_m d, mean_{b,m} min_n d)

Sharding: data-parallel over batch, 2 batches per core. Each core computes
its batches' sum-of-row-mins and sum-of-col-mins; host combines means.

Per-core algorithm (per batch of 32 n-chunks x [128, 4096] PSUM tiles):
  - Augmented K=7 float32r matmul produces distance tiles in PSUM:
      lhsT rows: [x~, y~, z~, s1h, s1l, 1, 1]  (x~ = f32r-rounded coords)
      rhs  rows: [-2x~', -2y~', -2z~', 1, 1, s2h, s2l]
    where sh = f32r(||rounded point||^2), sl = f32r(s - sh). Consistent
    rounding + hi/lo norm rows => the matmul yields the squared distance
    between the rounded points to ~1e-8, so no additive per-pair noise
    biases the min selection.
  - Engine split (cost-model-derived; per-elem costs from TimelineSim):
    the ACT engine drains each 4-bank PSUM tile to fp16 SBUF in one
    2048-wide activation copy (0.83 ns/elem; ACT is the only engine
    that can convert without stealing DVE cycles, and the instruction
    count is minimized because each op carries ~190 ns of fixed
    overhead). The DVE then does BOTH min directions from fp16 SBUF:
      dist1 (row-min): tensor_scalar op0=add 0 with op1=min accum_out
        runs in the 4x DVE perf mode for 2-byte all-SBUF operands
        (0.26 ns/elem); the accumulator column is the row min. (The 4x
        mode exists only for plain tensor_scalar - tensor_tensor /
        tensor_reduce / pool / InstMax all run at 2x or 1x.)
      dist2 (col-min): running tensor_tensor min into r (2x mode,
        0.52 ns/elem). Both ops are split per half-tile, which the
        pipeline scheduler prefers slightly.
    Steady state is ACT-bound at ~3.8 us/chunk with DVE at ~3.5; all
    attempts to shift drains onto DVE (whole chunks, halves, quarters)
    lose more to pipeline bubbles than they save on ACT.
  - Prep runs in a fat [48, 256] layout (engine cost is charged per
    free element, so 16x more partitions = 16x cheaper), with the norm
    row computed by a block-ones matmul (the neg2 side's 0.25 scale is
    folded into the block-ones constant), squares on the otherwise-idle
    GPSIMD engine, and DMAs spread across the SP/ACT HW queues + SWDGE
    so no single queue serializes the ~600 KB of prep traffic.
  - Finals: PE transposes of the running-min buffers + DVE reduces;
    sums via a ones-vector matmul.

Known-bad variants (measured, do not revisit): fp16 PSUM matmul output
(TRN3-only), GPSIMD min or PSUM access (unsupported on HW), matmul
free dim > 512 f32 (fails s3d3_mm_num_elements on HW), DMA-transpose
finals (+15 us in the cost model), PE warmup matmuls (+7..+26 us),
tensor_tensor_reduce (faults the device per prior session).
"""

import numpy as np
from contextlib import ExitStack

import concourse.bacc as bacc
import concourse.tile as tile
import concourse.mybir as mybir
from concourse import masks
from concourse import bass_utils

F32 = mybir.dt.float32
F32R = mybir.dt.float32r
F16 = mybir.dt.float16
MIN = mybir.AluOpType.min
MULT = mybir.AluOpType.mult
ADD = mybir.AluOpType.add
SUB = mybir.AluOpType.subtract
AX_X = mybir.AxisListType.X

P = 128          # partitions / n-chunk size
FREE = 512       # matmul free dim = one PSUM bank of fp32
HALF = 2048      # PSUM drain tile width (4 banks)
PREP_BLK = 256   # fat-prep block width

# Problem shape (hardcoded per contest contract)
B_FULL, N_PTS, M_PTS, D = 16, 4096, 4096, 3
N_CORES = 8
BPC = B_FULL // N_CORES  # batches per core

# Chunks (per batch) drained by DVE instead of ACT, to balance engines.
# Early chunks amortize the DVE deficit across the batch.
T3_CHUNKS = ()
# SBUF fp16 staging buffers
SPOOL_BUFS = 2

# Debug flags for timeline decomposition (leave False for production).
DBG_SKIP_FINALS = False
DBG_SKIP_PREP = False
DBG_SKIP_TS = False
DBG_SKIP_TT = False
DBG_SKIP_MM = False

# ACT drain ops per [128, HALF] PSUM tile (1, 2, or 4).
DRAIN_OPS = 1
# Every k-th chunk, DVE ts-drains one quarter straight from PSUM (0 = off).
DVE_Q_EVERY = 0
# PE warmup matmuls bridging prep -> mains (ramps the p-state model).
WARMUP_MMS = 0
# Batch-0 finals transposes on the (idle) DMA queues instead of PE/PSUM.
B0_DMA_TRANSPOSE = False
# Drain the very last half-tile on DVE to shorten the tail.
LAST_H1_DVE = False
# Split every chunk's col-min update into half-tile ops.
TT_SPLIT = True
# Split the row-min tensor_scalar per half as well.
TS_SPLIT = True
# Interleave batch-0's finals transpose groups into batch-1's mains,
# borrowing PSUM from the pm tile rotation (PE/DVE have per-chunk slack).
B0_INLINE_FINALS = False
# Route batch-1 input loads through the software DGE queue.
XF_B1_SWDGE = True


def _build(bpc, n, m, reps=1):
    """Build the SPMD program for `bpc` batches of [3, n] x [3, m] points."""
    nt_cnt = n // P            # n-chunks per batch
    blk = PREP_BLK             # fat-prep block width
    nb = n // blk              # fat-prep block count

    nc = bacc.Bacc("TRN2", target_bir_lowering=False, debug=False)
    x1d = nc.dram_tensor("x1", [bpc, D, n], F32, kind="ExternalInput")
    x2d = nc.dram_tensor("x2", [bpc, D, m], F32, kind="ExternalInput")
    onesd = nc.dram_tensor("ones", [2, max(n, m)], F32, kind="ExternalInput")
    bonesd = nc.dram_tensor("bones", [3 * nb, 2 * nb], F32,
                            kind="ExternalInput")
    outd = nc.dram_tensor("out", [2, bpc], F32, kind="ExternalOutput")

    with tile.TileContext(nc) as tc, ExitStack() as octx:
        consts = octx.enter_context(tc.tile_pool(name="consts", bufs=1))
        ones_p = consts.tile([P, 1], F32)
        nc.gpsimd.memset(ones_p[:], 1.0)
        ident16 = consts.tile([P, P], F16)
        masks.make_identity(nc, ident16[:])
        bonesb = consts.tile([3 * nb, 2 * nb], F32)
        nc.sync.dma_start(bonesb[:], bonesd[:])
        outsb = consts.tile([2, bpc], F32)

        for _rep in range(reps):
            with ExitStack() as ctx:
                apool = ctx.enter_context(tc.tile_pool(name="aug", bufs=1))
                accp = ctx.enter_context(tc.tile_pool(name="acc", bufs=1))
                rpool = ctx.enter_context(tc.tile_pool(name="R", bufs=1))

                # ---- prep: all batches, all sides, fat layout ----
                with ExitStack() as pctx:
                    scratch = pctx.enter_context(
                        tc.tile_pool(name="scratch", bufs=4))
                    afpp = pctx.enter_context(
                        tc.tile_pool(name="afpp", bufs=1))
                    ps_prep = pctx.enter_context(
                        tc.tile_pool(name="psp", bufs=4, space="PSUM"))

                    # stage all four sides' inputs and the afp ones-rows up
                    # front, spread across the SP/ACT HW queues + SWDGE so
                    # no single DMA queue serializes the prep
                    sides = [(b, sd) for b in range(bpc) for sd in range(2)]
                    xfs, afps = {}, {}
                    for i, (b, sd) in enumerate(sides):
                        xd = x2d if sd else x1d
                        cols = m if sd else n
                        xf = scratch.tile([D * nb, blk], F32,
                                          tag=f"xf{i}")
                        # batch-0 inputs on the HW queues; batch-1 via
                        # SWDGE so batch-0's assembly DMAs aren't queued
                        # behind batch-1's 48 KB loads
                        eng = (nc.sync, nc.scalar,
                               nc.gpsimd, nc.gpsimd)[i] if XF_B1_SWDGE \
                            else (nc.sync if i % 2 == 0 else nc.scalar)
                        eng.dma_start(
                            xf[:],
                            xd[b].rearrange("p (g c) -> (p g) c", c=blk))
                        xfs[(b, sd)] = xf
                        afp = afpp.tile([7, cols], F32, tag=f"afp{i}")
                        nc.gpsimd.dma_start(afp[3:7, :],
                                            onesd[0:1,
                                                  0:cols].broadcast_to(
                                                [4, cols]))
                        afps[(b, sd)] = afp

                    def prep_side(b, sd):
                        """Build the [7, cols] float32r augmented matrix."""
                        neg2 = bool(sd)
                        cols = m if sd else n
                        xf = xfs[(b, sd)]
                        afp = afps[(b, sd)]
                        # round (and scale by -2 for the rhs side); f32r of
                        # -2x equals -2 * f32r(x) exactly (power-of-two)
                        crf = scratch.tile([D * nb, blk], F32R, tag="crf")
                        if neg2:
                            nc.vector.tensor_scalar_mul(crf[:], xf[:], -2.0)
                        else:
                            nc.vector.tensor_copy(crf[:], xf[:])
                        # squares of the rounded coords on idle GPSIMD
                        sqf = scratch.tile([D * nb, blk], F32, tag="sqf")
                        nc.gpsimd.tensor_tensor(sqf[:], crf[:].bitcast(F32),
                                                crf[:].bitcast(F32), MULT)
                        # block-ones matmul sums the 3 coord rows per block;
                        # the neg2 columns carry 0.25 to undo the (-2)^2
                        pn = ps_prep.tile([nb, blk], F32, tag="pn")
                        bsl = bonesb[:, nb:2 * nb] if neg2 else bonesb[:, 0:nb]
                        nc.tensor.matmul(pn[:], bsl, sqf[:],
                                         start=True, stop=True)
                        # hi/lo split: sh = f32r(norm); lo = norm - sh
                        shf = scratch.tile([nb, blk], F32R, tag="shf")
                        nc.vector.tensor_copy(shf[:], pn[:])
                        lof = scratch.tile([nb, blk], F32, tag="lof")
                        nc.vector.tensor_tensor(lof[:], pn[:],
                                                shf[:].bitcast(F32), SUB)
                        # assemble the fp32 staging matrix via DMA; the fat
                        # tiles are p-major so their linear element order
                        # matches the destination rows exactly
                        r_norm = 5 if neg2 else 3
                        eng = nc.sync if sd == 0 else nc.scalar
                        eng.dma_start(afp[0:D, :], crf[:].bitcast(F32))
                        seng = (nc.sync if sd == 0 else nc.scalar) \
                            if b == 0 else nc.gpsimd
                        seng.dma_start(afp[r_norm:r_norm + 1, :],
                                       shf[:].bitcast(F32))
                        seng.dma_start(afp[r_norm + 1:r_norm + 2, :],
                                       lof[:])
                        # final rounding copy, split across ACT and DVE
                        A = apool.tile([7, cols], F32R,
                                       tag=f"A{b}_{int(neg2)}")
                        h = cols // 2
                        nc.scalar.copy(A[:, 0:h], afp[:, 0:h])
                        nc.vector.tensor_copy(A[:, h:cols], afp[:, h:cols])
                        if b == 0 and neg2:
                            # warmup matmuls on the staging data: keeps the
                            # PE busy through the A-round so the first real
                            # chunks run at full p-state
                            for i in range(WARMUP_MMS):
                                wu = ps_prep.tile([P, FREE], F32, tag="wu")
                                nc.tensor.matmul(
                                    wu[:], afp[:, 0:P], afp[:, 0:FREE],
                                    start=True, stop=True)
                        return A

                    def pe_warmup(cnt):
                        # dependency-light f16 junk matmuls bridging the
                        # prep phase into the mains so the PE p-state is
                        # hot when the first real chunks issue
                        if not cnt:
                            return
                        j16 = scratch.tile([P, FREE], F16, tag="j16")
                        nc.gpsimd.memset(j16[:], 1.0)
                        for _ in range(cnt):
                            wu = ps_prep.tile([P, FREE], F32, tag="wu")
                            nc.tensor.matmul(wu[:], j16[:, 0:P], j16[:],
                                             start=True, stop=True)

                    As = {}
                    if DBG_SKIP_PREP:
                        for b in range(bpc):
                            for sd in range(2):
                                A = apool.tile([7, n], F32R,
                                               tag=f"A{b}_{sd}")
                                nc.vector.memset(A[:], 1.0)
                                As[(b, sd)] = A
                    else:
                        for b in range(bpc):
                            As[(b, 0)] = prep_side(b, 0)
                            As[(b, 1)] = prep_side(b, 1)
                        pe_warmup(WARMUP_MMS)

                # ---- mains ----
                accs, rbufs, ssums = [], [], []
                with ExitStack() as mctx:
                    ps_main = mctx.enter_context(
                        tc.tile_pool(name="psm", bufs=2, space="PSUM"))
                    spool = mctx.enter_context(
                        tc.tile_pool(name="S", bufs=SPOOL_BUFS))
                    jpool = mctx.enter_context(
                        tc.tile_pool(name="junk", bufs=1))
                    junk = jpool.tile([P, m], F16)

                    acc2_pre = None
                    for b in range(bpc):
                        if b == 1 and B0_INLINE_FINALS \
                                and not DBG_SKIP_FINALS:
                            acc2_pre = accp.tile([P, m // P], F32,
                                                 tag="acc2p",
                                                 name="acc2_pre0")
                        A1, A2 = As[(b, 0)], As[(b, 1)]
                        r = rpool.tile([P, m], F16, tag=f"r{b}")
                        nc.vector.memset(r[:], 60000.0)
                        acc1 = accp.tile([P, 2 * nt_cnt], F32,
                                         tag=f"acc1_{b}")
                        nc.vector.memset(acc1[:], 3.0e38)

                        for nt in range(nt_cnt):
                            t3 = nt in T3_CHUNKS
                            dveq = DVE_Q_EVERY and (nt % DVE_Q_EVERY
                                                    == DVE_Q_EVERY - 1)
                            s = spool.tile([P, m], F16, tag="S", name="s")
                            for h in range(m // HALF):
                                pm = ps_main.tile([P, HALF], F32, tag="pm")
                                for j in range(HALF // FREE):
                                    mb = h * (HALF // FREE) + j
                                    nc.tensor.matmul(
                                        pm[:, j * FREE:(j + 1) * FREE],
                                        A1[:, nt * P:(nt + 1) * P],
                                        A2[:, mb * FREE:(mb + 1) * FREE],
                                        start=True, stop=True)
                                ssl = s[:, h * HALF:(h + 1) * HALF]
                                if nt == nt_cnt - 1 and h == 1 \
                                        and LAST_H1_DVE:
                                    # last half-drain on DVE so the tail
                                    # doesn't wait for ACT
                                    nc.vector.tensor_scalar(
                                        ssl, pm[:], 0.0, None,
                                        op0=ADD, op1=MIN,
                                        accum_out=acc1[:, 2 * nt + 1:
                                                       2 * nt + 2])
                                elif t3:
                                    # DVE drain with fused row-min accum
                                    nc.vector.tensor_scalar(
                                        ssl, pm[:], 0.0, None,
                                        op0=ADD, op1=MIN,
                                        accum_out=acc1[:, 2 * nt + h:
                                                       2 * nt + h + 1])
                                    continue
                                q0 = 0
                                if dveq and h == 0:
                                    # DVE drains the first quarter with
                                    # fused row-min accum
                                    qw = HALF // 2
                                    nc.vector.tensor_scalar(
                                        ssl[:, 0:qw], pm[:, 0:qw], 0.0,
                                        None, op0=ADD, op1=MIN,
                                        accum_out=acc1[:, 2 * nt + 1:
                                                       2 * nt + 2])
                                    q0 = qw
                                q = (HALF - q0) // DRAIN_OPS
                                for v in range(DRAIN_OPS):
                                    nc.scalar.copy(
                                        ssl[:, q0 + v * q:q0 + (v + 1) * q],
                                        pm[:, q0 + v * q:q0 + (v + 1) * q])
                            # batch-0 finals interleaved into batch-1's
                            # early chunks: 4 PE transposes + 1 DVE reduce
                            # per group, PSUM borrowed from the pm tile
                            # rotation (fits in the engines' slack)
                            if (b == 1 and B0_INLINE_FINALS
                                    and not DBG_SKIP_FINALS
                                    and nt % 4 == 2 and 2 <= nt < 31):
                                g = (nt - 2) // 4
                                tg0 = 4
                                ptile = ps_main.tile([P, HALF], F32,
                                                     tag="pm",
                                                     name=f"fin0_{g}")
                                pt16 = ptile[:, 0:tg0 * P // 2] \
                                    .bitcast(F16)
                                r0b = rbufs[0]
                                for k in range(tg0):
                                    tb = g * tg0 + k
                                    nc.tensor.transpose(
                                        pt16[:, k * P:(k + 1) * P],
                                        r0b[:, tb * P:(tb + 1) * P],
                                        ident16[:])
                                nc.vector.tensor_reduce(
                                    acc2_pre[:, g * tg0:(g + 1) * tg0],
                                    pt16[:].rearrange(
                                        "p (g c) -> p g c", c=P),
                                    axis=AX_X, op=MIN)
                            # running col-min first: the r-chain is the
                            # serial dependency across chunks; split the
                            # last chunk's update so finals start earlier
                            if not DBG_SKIP_TT:
                                if TT_SPLIT or nt == nt_cnt - 1:
                                    hm = m // 2
                                    nc.vector.tensor_tensor(
                                        r[:, 0:hm], s[:, 0:hm],
                                        r[:, 0:hm], MIN)
                                    nc.vector.tensor_tensor(
                                        r[:, hm:m], s[:, hm:m],
                                        r[:, hm:m], MIN)
                                else:
                                    nc.vector.tensor_tensor(r[:], s[:],
                                                            r[:], MIN)
                            if not t3 and not DBG_SKIP_TS:
                                # row-min from fp16 (4x DVE mode)
                                r0 = (HALF // 2) if dveq else 0
                                r1 = HALF if (nt == nt_cnt - 1
                                              and LAST_H1_DVE) else m
                                if TS_SPLIT and r0 == 0 and r1 == m:
                                    nc.vector.tensor_scalar(
                                        junk[:, 0:HALF], s[:, 0:HALF],
                                        0.0, None, op0=ADD, op1=MIN,
                                        accum_out=acc1[:, 2 * nt:
                                                       2 * nt + 1])
                                    nc.vector.tensor_scalar(
                                        junk[:, HALF:m], s[:, HALF:m],
                                        0.0, None, op0=ADD, op1=MIN,
                                        accum_out=acc1[:, 2 * nt + 1:
                                                       2 * nt + 2])
                                else:
                                    nc.vector.tensor_scalar(
                                        junk[:, r0:r1], s[:, r0:r1],
                                        0.0, None, op0=ADD, op1=MIN,
                                        accum_out=acc1[:, 2 * nt:
                                                       2 * nt + 1])
                        accs.append(acc1)
                        rbufs.append(r)

                # ---- finals ----
                if DBG_SKIP_FINALS:
                    continue
                with ExitStack() as fctx:
                    ps_tr = fctx.enter_context(
                        tc.tile_pool(name="pst", bufs=2, space="PSUM"))
                    tgrp = 4
                    trc = m // P
                    for b in range(bpc):
                        acc1, r = accs[b], rbufs[b]
                        if b == 0 and acc2_pre is not None:
                            acc2 = acc2_pre
                        else:
                            # all 32 transposes into one 4-bank f16 PSUM
                            # tile (free in the tail), then a single wide
                            # reduce (fewer per-op overheads and sems)
                            acc2 = accp.tile([P, trc], F32, tag="acc2",
                                             name=f"acc2_{b}")
                            hw2 = trc // 2
                            for half in range(2):
                                pt = ps_tr.tile([P, hw2 * P], F16,
                                                tag="pt")
                                for k in range(hw2):
                                    t = half * hw2 + k
                                    nc.tensor.transpose(
                                        pt[:, k * P:(k + 1) * P],
                                        r[:, t * P:(t + 1) * P],
                                        ident16[:])
                                nc.vector.tensor_reduce(
                                    acc2[:, half * hw2:(half + 1) * hw2],
                                    pt[:].rearrange("p (g c) -> p g c",
                                                    c=P),
                                    axis=AX_X, op=MIN)
                        # row-min: combine the per-chunk column pairs
                        d1 = accp.tile([P, nt_cnt], F32, tag="d1",
                                       name=f"d1_{b}")
                        nc.vector.tensor_reduce(
                            d1[:], acc1[:].rearrange("p (g c) -> p g c", c=2),
                            axis=AX_X, op=MIN)
                        ssum = accp.tile([P, 2], F32, tag=f"ssum_{b}")
                        nc.vector.tensor_reduce(ssum[:, 0:1], d1[:],
                                                axis=AX_X, op=ADD)
                        nc.vector.tensor_reduce(ssum[:, 1:2], acc2[:],
                                                axis=AX_X, op=ADD)
                        po = ps_tr.tile([2, 1], F32, tag="po")
                        nc.tensor.matmul(po[:], ssum[:], ones_p[:],
                                         start=True, stop=True)
                        nc.scalar.copy(outsb[:, b:b + 1], po[:])

        nc.sync.dma_start(outd[:], outsb[:])

    nc.compile()
    return nc


_NC_CACHE = {}


def _get_nc():
    key = (BPC, N_PTS, M_PTS)
    if key not in _NC_CACHE:
        _NC_CACHE[key] = _build(*key)
    return _NC_CACHE[key]


def _bones(nb):
    # fat layout is p-major: partition k = d*nb + g belongs to block g
    b = np.zeros((3 * nb, 2 * nb), dtype=np.float32)
    for d in range(3):
        for g in range(nb):
            b[d * nb + g, g] = 1.0
            b[d * nb + g, nb + g] = 0.25
    return b


def run(xyz1, xyz2, trace=False):
    """Run on 8 cores; returns ((mean1, mean2), exec_time_ns_or_None)."""
    x1 = np.ascontiguousarray(
        np.asarray(xyz1, dtype=np.float32).transpose(0, 2, 1))  # [B, 3, N]
    x2 = np.ascontiguousarray(
        np.asarray(xyz2, dtype=np.float32).transpose(0, 2, 1))  # [B, 3, M]
    assert x1.shape == (B_FULL, D, N_PTS) and x2.shape == (B_FULL, D, M_PTS)

    nc = _get_nc()
    ones_row = np.ones((2, max(N_PTS, M_PTS)), dtype=np.float32)
    bones = _bones(N_PTS // PREP_BLK)
    in_maps = [
        {"x1": np.ascontiguousarray(x1[c * BPC:(c + 1) * BPC]),
         "x2": np.ascontiguousarray(x2[c * BPC:(c + 1) * BPC]),
         "ones": ones_row,
         "bones": bones}
        for c in range(N_CORES)
    ]
    res = bass_utils.run_bass_kernel_spmd(nc, in_maps, list(range(N_CORES)),
                                          trace=trace)
    sum1 = 0.0
    sum2 = 0.0
    for c in range(N_CORES):
        o = np.asarray(res.results[c]["out"], dtype=np.float64)
        sum1 += o[0].sum()
        sum2 += o[1].sum()
    mean1 = np.float32(sum1 / (B_FULL * N_PTS))
    mean2 = np.float32(sum2 / (B_FULL * M_PTS))
    return (mean1, mean2), res.exec_time_ns


def kernel(xyz1, xyz2):
    return run(xyz1, xyz2, trace=False)[0]
